# revision 1
# baseline (speedup 1.0000x reference)
"""SlimMambaBlock Trainium2 kernel (v2).

Full-input contract: kernel(**inputs) takes the complete tensors
(x [8, 4096, 256], norm_w [256], W_in [1024, 256], W_dt [512, 512],
b_dt [512], W_out [256, 512]) and returns the full output [8, 4096, 256].

Sharding: data-parallel over batch — core b processes batch b (8 cores).

Per-core program (Tile framework), feature-major activation layout,
512-token chunks, deep software pipeline (x DMA 5 cycles ahead, RMS
stats/Newton/h-scale 4 ahead, PE transposes 2 ahead at the PE queue
front, every producer one full cycle before its consumer):

  1. One wide DMA per chunk: xin [128, 4*256] (token-tile-major).
     RMS stats split ACT (Square+accum) / DVE (scalar_tensor_tensor
     (x+0)*x with accum_out; NB tensor_tensor_reduce hangs real HW).
     inv_rms via Newton-rsqrt: bit-trick seed on DVE, iterations on
     Pool (keeps ScalarE on ONE activation table: silu_and_others =
     {Silu, Tanh, Square, Copy}; table reloads are 1.3us).
  2. h = x*inv_rms on Pool (early in its queue each cycle), then
     PE-transpose h -> hT [d, tok]; copies PSUM->SBUF bf16 on DVE.
  3. in_proj: uvT[feat, tok] = W_inT.T @ hT ; u = silu, g = silu (ACT).
     The c+2 transposes ride inside in_proj's silu-drain stalls.
  4. dt_proj: preT = W_dtT.T @ uT ; th = tanh(pre/2 + b_dt/2) (ACT)
     lam = sigmoid(pre + b_dt) = 0.5*th + 0.5 (DVE tensor_scalar, 4x)
  5. b = (th-1)*u: tm = th-1 (DVE tensor_scalar 4x), b = tm*u (DVE
     tensor_mul, bf16 2x). All on DVE: the cross-engine tm->Pool->scan
     hop was the scan's critical-path bottleneck.
  6. recurrence via DVE tensor_tensor_scan with S = -2*s:
     S_t = lam_t * S_{t-1} + b_t; sg = S*g (bf16 tensor_mul; the -0.5
     of s = -S/2 is folded into W_outT at weight prep).
  7. out_proj with sgT stationary -> y [128,512] token-pair PSUM tiles
     (a single accumulation group per pair: one start zeroes the bank,
     each half's first write lands on zeros); pair 0 is emitted between
     in_proj(c+1) and dt_proj(c+1) so its residual clears the PSUM
     bank before pair 1 needs it.
  8. resid ow = y + x (DVE, PSUM+SBUF); one wide store per chunk
     (split per-pair on the last chunk to shorten the drain tail).

Matmul operands are bf16 (PE 1 cycle/row vs 4 for fp32). Weights are
loaded with wide strided DMAs and PE-transposed once at startup,
interleaved with the x-chunk lookahead loads. The scan state is fp32
internally; RMS stats, Newton, and the residual add stay fp32.
Measured rel err ~8.6e-4 (HW); TimelineSim 104.7us vs 151.1us for the
previous version of this kernel.
"""

import numpy as np

B, K, D = 8, 4096, 256
INNER = 512
EPS = 1e-5
TC = 512                 # tokens per chunk
NCHUNK = K // TC         # 8
NTT = TC // 128          # token-tiles per chunk

N_CORES = 8
MAGIC = 0x5F3759DF       # fast inverse sqrt seed

_CACHE: dict = {}


def _emit(tc, aps, mm_f32r=True, silu_native=True):
    """Emit the per-core program. aps: dict of DRAM APs."""
    import concourse.bass as bass
    import concourse.mybir as mybir
    from concourse import masks

    nc = tc.nc
    f32 = mybir.dt.float32
    i32 = mybir.dt.int32
    AF = mybir.ActivationFunctionType
    ALU = mybir.AluOpType
    ts = bass.ts

    # All matmul operands bf16 (PE 1 cycle/row vs 4 for fp32).
    fr = mybir.dt.bfloat16 if mm_f32r else f32

    def pe_transpose(out_ap, in_ap, stat_ap):
        nc.tensor.matmul(out_ap, in_ap, stat_ap, is_transpose=True)

    x_d = aps["x"]
    nw_d = aps["norm_w"]
    win_d = aps["W_in"]
    wdt_d = aps["W_dt"]
    bdt_d = aps["b_dt"]
    wout_d = aps["W_out"]
    out_d = aps["out"]

    import contextlib
    ctx = contextlib.ExitStack()
    with ctx:
        const = ctx.enter_context(tc.tile_pool(name="const", bufs=1))
        wraw = ctx.enter_context(tc.tile_pool(name="wraw", bufs=1))
        wT = ctx.enter_context(tc.tile_pool(name="wT", bufs=1))
        xinp = ctx.enter_context(tc.tile_pool(name="xinp", bufs=7))
        sqp = ctx.enter_context(tc.tile_pool(name="sqp", bufs=2))
        sigp = ctx.enter_context(tc.tile_pool(name="sigp", bufs=2))
        statp = ctx.enter_context(tc.tile_pool(name="statp", bufs=10))
        hwp = ctx.enter_context(tc.tile_pool(name="hwp", bufs=4))
        hTp = ctx.enter_context(tc.tile_pool(name="hTp", bufs=3))
        uTp = ctx.enter_context(tc.tile_pool(name="uTp", bufs=3))
        gTp = ctx.enter_context(tc.tile_pool(name="gTp", bufs=3))
        thTp = ctx.enter_context(tc.tile_pool(name="thTp", bufs=3))
        lamTp = ctx.enter_context(tc.tile_pool(name="lamTp", bufs=3))
        bTp = ctx.enter_context(tc.tile_pool(name="bTp", bufs=3))
        sTp = ctx.enter_context(tc.tile_pool(name="sTp", bufs=3))
        sgTp = ctx.enter_context(tc.tile_pool(name="sgTp", bufs=3))
        owp = ctx.enter_context(tc.tile_pool(name="owp", bufs=3))

        # PSUM budget (8 banks): uvps [128,512] x2 = 2, preps [128,512]
        # x2 = 2, tps [128,512] x2 = 2, yps [128,256] x2 = 2.
        tps = ctx.enter_context(tc.tile_pool(name="tps", bufs=2, space="PSUM"))
        uvps = ctx.enter_context(tc.tile_pool(name="uvps", bufs=2, space="PSUM"))
        preps = ctx.enter_context(tc.tile_pool(name="preps", bufs=2, space="PSUM"))
        yps = ctx.enter_context(tc.tile_pool(name="yps", bufs=2, space="PSUM"))

        # ---- constants ----
        identf = const.tile([128, 128], f32, tag="identf", name="identf")
        masks.make_identity(nc, identf[:])
        magic = const.tile([128, NTT], i32, tag="magic", name="magic")
        nc.gpsimd.memset(magic[:], MAGIC)

        nw, bdt2 = [], []

        def load_vec_consts():
            bw = const.tile([128, 4], f32, tag="bdtw", name="bdtw")
            nc.sync.dma_start(bw[:], bdt_d.rearrange("(m p) -> p m", p=128))
            # scale in place: tanh(pre*0.5 + b_dt*0.5)
            nc.vector.tensor_scalar_mul(bw[:], bw[:], 0.5)
            for m in range(4):
                bdt2.append(bw[:, m:m + 1])

        # ---- load + transpose weights ----
        winT, wdtT, woutT = [], [], []
        wtiles = {}

        def win_dma():
            for k in range(2):
                t = const.tile([128, 1], f32, tag=f"nw{k}", name=f"nw{k}")
                nc.sync.dma_start(
                    t[:], nw_d[ts(k, 128)].rearrange("(a b) -> a b", b=1))
                nw.append(t)
            winw = wraw.tile([128, 8 * 256], f32, tag="winw", name="winw")
            for hf in range(2):
                nc.sync.dma_start(
                    winw[:, hf * 1024:(hf + 1) * 1024].rearrange(
                        "p (f d) -> p f d", f=4),
                    win_d[hf * 512:(hf + 1) * 512, :].rearrange(
                        "(f p) d -> p f d", p=128))
            wtiles["winw"] = winw

        def win_transform():
            # W_in [1024(feat), 256(d)] -> W_inT [2][128(d), 1024(feat)]
            winw = wtiles["winw"]
            win_raw = [winw[:, f * 256:(f + 1) * 256] for f in range(8)]
            for k in range(2):
                t = wT.tile([128, 1024], fr, tag=f"winT{k}", name=f"winT{k}")
                winT.append(t)
            for half in range(2):
                for k in range(2):
                    p = tps.tile([128, 512], f32, tag="tp", name="tpw")
                    for j in range(4):
                        f = half * 4 + j
                        pe_transpose(p[:, ts(j, 128)],
                                     win_raw[f][:, ts(k, 128)], identf[:])
                    if half % 2 == 0:
                        nc.vector.tensor_copy(winT[k][:, ts(half, 512)], p[:])
                    else:
                        nc.scalar.copy(winT[k][:, ts(half, 512)], p[:])
            for k in range(2):
                # fold norm_w (per-d row scale) into W_inT
                nc.vector.tensor_scalar_mul(winT[k][:], winT[k][:], nw[k][:])

        def wdt_dma():
            wdtw = wraw.tile([128, 4 * 512], f32, tag="wdtw", name="wdtw")
            nc.sync.dma_start(
                wdtw[:].rearrange("p (m e) -> p m e", m=4),
                wdt_d.rearrange("(m p) e -> p m e", p=128))
            wtiles["wdtw"] = wdtw

        def wdt_transform():
            # W_dt [512(e_out), 512(e_in)] -> W_dtT [4][128(e_in), 512(e_out)]
            wdtw = wtiles["wdtw"]
            wdt_raw = [wdtw[:, m * 512:(m + 1) * 512] for m in range(4)]
            for k in range(4):
                t = wT.tile([128, 512], fr, tag=f"wdtT{k}", name=f"wdtT{k}")
                wdtT.append(t)
            for k in range(4):
                p = tps.tile([128, 512], f32, tag="tp", name="tpw")
                for m in range(4):
                    pe_transpose(p[:, ts(m, 128)], wdt_raw[m][:, ts(k, 128)],
                                 identf[:])
                if k % 2 == 0:
                    nc.vector.tensor_copy(wdtT[k][:], p[:])
                else:
                    nc.scalar.copy(wdtT[k][:], p[:])

        def wout_dma():
            woutw = wraw.tile([128, 2 * 512], f32, tag="woutw", name="woutw")
            nc.sync.dma_start(
                woutw[:].rearrange("p (a e) -> p a e", a=2),
                wout_d.rearrange("(a p) e -> p a e", p=128))
            wtiles["woutw"] = woutw

        def wout_transform():
            # W_out [256(d), 512(e)] -> W_outT [4][128(e), 256(d)]
            woutw = wtiles["woutw"]
            wout_raw = [woutw[:, a * 512:(a + 1) * 512] for a in range(2)]
            for e in range(4):
                t = wT.tile([128, 256], fr, tag=f"woutT{e}", name=f"woutT{e}")
                woutT.append(t)
            for e in range(4):
                p = tps.tile([128, 512], f32, tag="tp", name="tpw")
                for dd in range(2):
                    pe_transpose(p[:, ts(dd, 128)],
                                 wout_raw[dd][:, ts(e, 128)], identf[:])
                # fold the S = -2*s change of variable into W_out: scale by
                # -0.5 so sg = S*g needs no extra scaling (plain tensor_mul)
                if e % 2 == 0:
                    nc.vector.tensor_scalar_mul(woutT[e][:], p[:, :256], -0.5)
                else:
                    nc.scalar.mul(woutT[e][:], p[:, :256], -0.5)

        # ---- pipeline stages ----
        def dma_issue(c):
            """Issue the wide x load for chunk c (4 cycles ahead)."""
            xin = xinp.tile([128, NTT * D], f32, tag="xin", name="xin")
            src = x_d[c * TC:(c + 1) * TC, :].rearrange(
                "(t p) d -> p t d", p=128)
            nc.sync.dma_start(
                xin[:].rearrange("p (t d) -> p t d", t=NTT), src)
            return xin

        def norm_calc(st):
            """RMS stats (ACT/DVE queue fronts), Newton-rsqrt + h = x*inv_rms
            (all on Pool: one cross-engine hop, not the DVE conveyor)."""
            xin = st["xin"]
            vpk = statp.tile([128, NTT], f32, tag="vpk", name="vpk")
            for t in range(NTT):
                xv = xin[:, t * D:(t + 1) * D]
                sq = sqp.tile([128, D], f32, tag="sq", name="sq")
                if t % 2 == 0:
                    nc.scalar.activation(sq[:], xv, AF.Square,
                                         accum_out=vpk[:, t:t + 1])
                else:
                    # x^2 + free-axis sum on DVE: scalar_tensor_tensor
                    # (x + 0) * x with accum_out (tensor_tensor_reduce
                    # hangs real hardware; stt is HW-proven)
                    nc.vector.scalar_tensor_tensor(
                        sq[:], xv, 0.0, xv, op0=ALU.add, op1=ALU.mult,
                        accum_out=vpk[:, t:t + 1])

            # inv_rms = rsqrt(vpk/D + eps): nv + bit-trick seed on DVE
            nv = statp.tile([128, NTT], f32, tag="nv", name="nv")
            nc.vector.tensor_scalar(nv[:], vpk[:], 1.0 / D, EPS,
                                    op0=ALU.mult, op1=ALU.add)
            ny = statp.tile([128, NTT], f32, tag="ny", name="ny")
            # seed: y0 = bits(magic - (bits(v) >> 1))
            nyi = ny[:].bitcast(i32)
            nc.vector.tensor_scalar(nyi, nv[:].bitcast(i32), 1, None,
                                    op0=ALU.arith_shift_right)
            nc.vector.scalar_tensor_tensor(nyi, magic[:], 1, nyi,
                                           op0=ALU.bypass, op1=ALU.subtract)
            nt = statp.tile([128, NTT], f32, tag="nt", name="nt")
            for _ in range(3):
                # t = v*y*y ; y = y * (1.5 - 0.5*t)
                nc.gpsimd.tensor_mul(nt[:], ny[:], ny[:])
                nc.gpsimd.tensor_mul(nt[:], nt[:], nv[:])
                nc.gpsimd.tensor_scalar(nt[:], nt[:], -0.5, 1.5,
                                        op0=ALU.mult, op1=ALU.add)
                nc.gpsimd.tensor_mul(ny[:], ny[:], nt[:])

            hw_ = hwp.tile([128, NTT * D], f32, tag="hw", name="hw")
            for t in range(NTT):
                nc.gpsimd.tensor_scalar_mul(
                    hw_[:, t * D:(t + 1) * D], xin[:, t * D:(t + 1) * D],
                    ny[:, t:t + 1])
            st["hw"] = hw_

        def norm_transpose_k(st, k):
            """PE-transpose h d-half k -> PSUM (interleaved into in_proj's
            silu-drain stalls). The PSUM->SBUF copies are emitted separately
            (norm_copies) so the scans stay at the DVE front."""
            hw_ = st["hw"]
            p = tps.tile([128, TC], f32, tag="tp", name="tp")
            for t in range(NTT):
                pe_transpose(p[:, ts(t, 128)],
                             hw_[:, t * D + k * 128:t * D + k * 128 + 128],
                             identf[:])
            st.setdefault("hTps", []).append(p)

        def norm_copies(st):
            hT = [hTp.tile([128, TC], fr, tag=f"hT{k}", name=f"hT{k}")
                  for k in range(2)]
            for k in range(2):
                nc.vector.tensor_copy(hT[k][:], st["hTps"][k][:])
            st["hT"] = hT

        def front_in(st):
            """in_proj + silu (+ interleaved c+2 transposes)."""
            hT = st["hT"]
            uTw = [uTp.tile([128, 2 * TC], fr, tag=f"uTw{p}", name=f"uTw{p}")
                   for p in range(2)]
            gTw = [gTp.tile([128, 2 * TC], fr, tag=f"gTw{p}", name=f"gTw{p}")
                   for p in range(2)]
            uT = [uTw[m // 2][:, (m % 2) * TC:(m % 2 + 1) * TC]
                  for m in range(4)]
            gT = [gTw[m // 2][:, (m % 2) * TC:(m % 2 + 1) * TC]
                  for m in range(4)]
            st_T = st.get("st_T")
            for m in range(8):
                ps = uvps.tile([128, TC], f32, tag="uv", name="uv")
                for k in range(2):
                    nc.tensor.matmul(
                        ps[:], winT[k][:, ts(m, 128)], hT[k][:],
                        start=(k == 0), stop=(k == 1),
                    )
                dst = uT[m] if m < 4 else gT[m - 4]
                if silu_native:
                    nc.scalar.activation(dst, ps[:], AF.Silu)
                else:
                    # CoreSim has no Silu: decompose as x * sigmoid(x)
                    sig = sigp.tile([128, TC], f32, tag="sig", name="sig")
                    nc.scalar.activation(sig[:], ps[:], AF.Sigmoid)
                    nc.vector.tensor_mul(dst, ps[:], sig[:])
                # fill PE's silu-drain wait with the c+2 transposes
                if st_T is not None and m in (1, 3):
                    norm_transpose_k(st_T, m // 2)
            st.update(uT=uT, gT=gT, gTw=gTw)

        def front_dt(st):
            """dt_proj + tanh + lam + b."""
            uT = st["uT"]
            thT = [thTp.tile([128, TC], fr, tag=f"thT{m}", name=f"thT{m}")
                   for m in range(4)]
            lamT = [lamTp.tile([128, TC], fr, tag=f"lamT{m}", name=f"lamT{m}")
                    for m in range(4)]
            bT = [bTp.tile([128, TC], fr, tag=f"bT{m}", name=f"bT{m}")
                  for m in range(4)]
            for m in range(4):
                ps = preps.tile([128, TC], f32, tag="pre", name="pre")
                for k in range(4):
                    nc.tensor.matmul(
                        ps[:], wdtT[k][:, ts(m, 128)], uT[k],
                        start=(k == 0), stop=(k == 3),
                    )
                nc.scalar.activation(thT[m][:], ps[:], AF.Tanh,
                                     bias=bdt2[m], scale=0.5)
                nc.vector.tensor_scalar(lamT[m][:], thT[m][:], 0.5, 0.5,
                                        op0=ALU.mult, op1=ALU.add)
                # b = (th - 1) * u: tm on DVE (tensor_scalar, 4x mode),
                # the multiply on Pool (keeps DVE under the PE roofline).
                # walrus rejects scalar_tensor_tensor on Pool.
                tm = bTp.tile([128, TC], fr, tag="tm", name="tm")
                nc.vector.tensor_scalar(tm[:], thT[m][:], 1.0, None,
                                        op0=ALU.subtract)
                nc.vector.tensor_mul(bT[m][:], tm[:], uT[m])
            st.update(lamT=lamT, bT=bT)

        def scan_stage(st, sT_prev):
            # scan (DVE): S_t = lam_t*S_{t-1} + bT_t  => S = -2*s
            sTw = [sTp.tile([128, 2 * TC], fr, tag=f"sTw{p}",
                            name=f"sTw{p}") for p in range(2)]
            sgTw = [sgTp.tile([128, 2 * TC], fr, tag=f"sgTw{p}",
                              name=f"sgTw{p}") for p in range(2)]
            sT = [sTw[m // 2][:, (m % 2) * TC:(m % 2 + 1) * TC]
                  for m in range(4)]
            sgT = [sgTw[m // 2][:, (m % 2) * TC:(m % 2 + 1) * TC]
                   for m in range(4)]
            for m in range(4):
                init = 0.0 if sT_prev is None else sT_prev[m][:, TC - 1:TC]
                nc.vector.tensor_tensor_scan(
                    sT[m], st["lamT"][m][:], st["bT"][m][:], init,
                    op0=ALU.mult, op1=ALU.add,
                )
                # sg = S*g (the -0.5 of s = -S/2 is folded into woutT),
                # one wide op per m-pair (scan m+1 of the pair has slack
                # before out_proj consumes sg)
                if m % 2 == 1:
                    pr = m // 2
                    gw = st["gTw"][pr]
                    nc.vector.tensor_mul(sgTw[pr][:], sTw[pr][:], gw[:])
            st.update(sT=sT, sgT=sgT)
            return sT

        def out_pair(st, c, pair, split_store=False):
            # out_proj (sgT stationary -> y [tok,d] tiles) + residual into
            # the wide out tile. Pair 0 is emitted mid-cycle (between
            # in_proj and dt_proj of c+1) so its residual clears the yps
            # bank before pair 1's matmuls need it.
            sgT, xin = st["sgT"], st["xin"]
            if pair == 0:
                st["ow"] = owp.tile([128, NTT * D], f32, tag="ow", name="ow")
            ow = st["ow"]
            tt = (2 * pair, 2 * pair + 1)
            # one [128,512] PSUM tile per pair, single accumulation group
            # (one start zeroes the bank; each first write lands on zeros)
            yp = yps.tile([128, 2 * D], f32, tag="y", name="y")
            for e in range(4):
                for i, t in enumerate(tt):
                    nc.tensor.matmul(
                        yp[:, i * D:(i + 1) * D],
                        sgT[e][:, ts(t, 128)], woutT[e][:],
                        start=(e == 0 and i == 0), stop=(e == 3 and i == 1),
                    )
            nc.vector.tensor_add(
                ow[:, 2 * pair * D:(2 * pair + 2) * D], yp[:],
                xin[:, 2 * pair * D:(2 * pair + 2) * D])
            if split_store:
                t0 = c * TC + pair * 256
                dst = out_d[t0:t0 + 256, :].rearrange(
                    "(t p) d -> p t d", p=128)
                nc.sync.dma_start(
                    dst, ow[:, pair * 512:(pair + 1) * 512].rearrange(
                        "p (t d) -> p t d", t=2))

        def out_store(st, c):
            dst = out_d[c * TC:(c + 1) * TC, :].rearrange(
                "(t p) d -> p t d", p=128)
            nc.sync.dma_start(
                dst, st["ow"][:].rearrange("p (t d) -> p t d", t=NTT))

        # Software-pipelined emission: every producer finishes one full
        # cycle before its consumer. x DMAs 4 cycles ahead, stats/Newton/
        # h-scale 3 ahead, transposes 2 ahead (PE queue front).
        sts = [dict() for _ in range(NCHUNK)]
        # Preamble: x0 lands first (its stats chain is the longest pole),
        # weight DMAs interleave with the x lookahead loads.
        sts[0]["xin"] = dma_issue(0)
        win_dma()
        sts[1]["xin"] = dma_issue(1)
        wdt_dma()
        sts[2]["xin"] = dma_issue(2)
        wout_dma()
        sts[3]["xin"] = dma_issue(3)
        win_transform()
        norm_calc(sts[0])
        wdt_transform()
        wout_transform()
        norm_transpose_k(sts[0], 0)
        norm_transpose_k(sts[0], 1)
        norm_copies(sts[0])
        norm_calc(sts[1])
        norm_transpose_k(sts[1], 0)
        norm_transpose_k(sts[1], 1)
        norm_copies(sts[1])
        sts[4]["xin"] = dma_issue(4)
        load_vec_consts()
        norm_calc(sts[2])
        norm_calc(sts[3])
        sts[0]["st_T"] = sts[2]
        front_in(sts[0])
        front_dt(sts[0])
        norm_copies(sts[2])
        sT_prev = None
        for c in range(NCHUNK):
            if c + 4 < NCHUNK:
                norm_calc(sts[c + 4])
            sT_prev = scan_stage(sts[c], sT_prev)
            if c + 5 < NCHUNK:
                sts[c + 5]["xin"] = dma_issue(c + 5)
            last = c == NCHUNK - 1
            if c + 1 < NCHUNK:
                # transposes for c+3 ride inside front(c+1)'s stall slots
                if c + 3 < NCHUNK:
                    sts[c + 1]["st_T"] = sts[c + 3]
                front_in(sts[c + 1])
                out_pair(sts[c], c, 0, split_store=last)
                front_dt(sts[c + 1])
                out_pair(sts[c], c, 1, split_store=last)
                if c + 3 < NCHUNK:
                    norm_copies(sts[c + 3])
            else:
                out_pair(sts[c], c, 0, split_store=True)
                out_pair(sts[c], c, 1, split_store=True)
            if not last:
                out_store(sts[c], c)
            sts[c].clear()


def build(mm_f32r=True, silu_native=True, **emit_kw):
    """Build and compile the Bass module (cached)."""
    key = ("nc", mm_f32r, silu_native, tuple(sorted(emit_kw.items())))
    if key in _CACHE:
        return _CACHE[key]

    from concourse import bacc, mybir, tile

    f32 = mybir.dt.float32
    nc = bacc.Bacc(
        "TRN2",
        target_bir_lowering=False,
        debug=False,
        num_devices=N_CORES,
    )
    aps = {
        "x": nc.dram_tensor("x", [K, D], f32, kind="ExternalInput").ap(),
        "norm_w": nc.dram_tensor("norm_w", [D], f32, kind="ExternalInput").ap(),
        "W_in": nc.dram_tensor("W_in", [2 * INNER, D], f32, kind="ExternalInput").ap(),
        "W_dt": nc.dram_tensor("W_dt", [INNER, INNER], f32, kind="ExternalInput").ap(),
        "b_dt": nc.dram_tensor("b_dt", [INNER], f32, kind="ExternalInput").ap(),
        "W_out": nc.dram_tensor("W_out", [D, INNER], f32, kind="ExternalInput").ap(),
        "out": nc.dram_tensor("out", [K, D], f32, kind="ExternalOutput").ap(),
    }
    with tile.TileContext(nc) as tc:
        _emit(tc, aps, mm_f32r=mm_f32r, silu_native=silu_native, **emit_kw)
    nc.compile()
    _CACHE[key] = nc
    return nc


def make_in_maps(inputs):
    x = np.asarray(inputs["x"], dtype=np.float32)
    shared = {
        "norm_w": np.asarray(inputs["norm_w"], dtype=np.float32),
        "W_in": np.asarray(inputs["W_in"], dtype=np.float32),
        "W_dt": np.asarray(inputs["W_dt"], dtype=np.float32),
        "b_dt": np.asarray(inputs["b_dt"], dtype=np.float32),
        "W_out": np.asarray(inputs["W_out"], dtype=np.float32),
    }
    return [
        {"x": np.ascontiguousarray(x[b]), **shared} for b in range(N_CORES)
    ]


def run(inputs, trace=False, mm_f32r=True, silu_native=True, **kw):
    from concourse.bass_utils import run_bass_kernel_spmd

    nc = build(mm_f32r=mm_f32r, silu_native=silu_native)
    in_maps = make_in_maps(inputs)
    res = run_bass_kernel_spmd(
        nc, in_maps, core_ids=list(range(N_CORES)), trace=trace, **kw
    )
    out = np.stack([res.results[b]["out"] for b in range(N_CORES)], axis=0)
    return out, res


def kernel(**inputs) -> np.ndarray:
    out, _ = run(inputs, trace=False)
    return out



# revision 26
# speedup vs baseline: 15.6556x; 15.6556x over previous
"""SlimMambaBlock Trainium2 kernel (v3).

Full-input contract: kernel(**inputs) takes the complete tensors
(x [8, 4096, 256], norm_w [256], W_in [1024, 256], W_dt [512, 512],
b_dt [512], W_out [256, 512]) and returns the full output [8, 4096, 256].

Sharding: data-parallel over batch — core b processes batch b (8 cores).

v3 (from HW ablations: gpsimd/Pool tensor ops cost ~100us/iter of the
measured ~208us steady-state — 4-5x the cost model — so the Q7 path is
evicted from the steady state entirely; DVE is the #2 limiter, ACT has
headroom, DMA is a non-issue):

  1. x loaded as bf16 (SWDGE cast-DMA, 2 wide loads), all 8 chunks
     upfront. RMS stats batched: per token-tile Square+accum (ACT/DVE
     split) into one vpk[128, 32]; ONE Newton-rsqrt chain on DVE
     (bit-trick seed + 2 iterations) -> ny[128, 32] for the whole pass.
  2. h-scale is FUSED into the PE transpose: instead of transposing h
     by identity, transpose x by diag(inv_rms): xT@diag gives
     hT[d, tok] = x[tok, d]*ny[tok] in the same PE instruction count.
     diag tiles are built on ACT (Copy activation, per-partition scale
     applied to a bf16 identity). No gpsimd, no separate h tile.
  3. in_proj: uvT[feat, tok] = W_inT.T @ hT ; u = silu, g = silu (ACT).
     (norm_w is folded into W_inT rows at weight prep.)
  4. dt_proj: preT = W_dtT.T @ uT ; th = tanh(pre/2 + b_dt/2) (ACT);
     lam = sigmoid(pre + b_dt) = 0.5*th + 0.5 (DVE tensor_scalar);
     b = (th - 1)*u in ONE DVE scalar_tensor_tensor op.
  5. recurrence via DVE tensor_tensor_scan with S = -2*s:
     S_t = lam_t * S_{t-1} + b_t; sg = S*g (bf16 tensor_mul; the -0.5
     of s = -S/2 is folded into W_outT at weight prep).
  6. out_proj with sgT stationary -> y [128,512] token-pair PSUM tiles;
     resid ow = y + x (DVE, PSUM f32 + SBUF bf16 -> f32); one wide
     fp32 store per chunk (split per-pair on the last chunk).

All matmul operands bf16 (PE 1 cycle/row). Both transpose d-halves
share one [128, 2*TC] PSUM tile -> one wide PSUM->SBUF copy per chunk
(DVE). The scan state is fp32 internal to the scan instruction; RMS
stats/Newton stay fp32. x in bf16 costs ~6e-4 output rel err (budget
2e-2; measured total 1.97e-3).

Measured (chained-donation R-slope, axon trn2): ~100-105us steady-state
per execution vs ~208us for the v2 baseline; TimelineSim models 103us
single-exec + ~12us startup. Engine balance (modeled): ACT ~79 / PE ~76
/ DVE ~75us.
"""

import numpy as np

B, K, D = 8, 4096, 256
INNER = 512
EPS = 1e-5
TC = 512                 # tokens per chunk
NCHUNK = K // TC         # 8
NTT = TC // 128          # token-tiles per chunk
NTILE = NCHUNK * NTT     # 32 token-tiles per pass

N_CORES = 8
MAGIC = 0x5F3759DF       # fast inverse sqrt seed

_CACHE: dict = {}


def _emit(tc, aps, mm_f32r=True, silu_native=True):
    """Emit the per-core program. aps: dict of DRAM APs."""
    import concourse.bass as bass
    import concourse.mybir as mybir
    from concourse import masks

    nc = tc.nc
    f32 = mybir.dt.float32
    i32 = mybir.dt.int32
    AF = mybir.ActivationFunctionType
    ALU = mybir.AluOpType
    ts = bass.ts

    # All matmul operands bf16 (PE 1 cycle/row vs 4 for fp32).
    fr = mybir.dt.bfloat16 if mm_f32r else f32

    x_d = aps["x"]
    nw_d = aps["norm_w"]
    win_d = aps["W_in"]
    wdt_d = aps["W_dt"]
    bdt_d = aps["b_dt"]
    wout_d = aps["W_out"]
    out_d = aps["out"]

    import contextlib
    ctx = contextlib.ExitStack()
    with ctx:
        const = ctx.enter_context(tc.tile_pool(name="const", bufs=1))
        wraw = ctx.enter_context(tc.tile_pool(name="wraw", bufs=1))
        wT = ctx.enter_context(tc.tile_pool(name="wT", bufs=1))
        xinp = ctx.enter_context(tc.tile_pool(name="xinp", bufs=6))
        sqp = ctx.enter_context(tc.tile_pool(name="sqp", bufs=2))
        sigp = ctx.enter_context(tc.tile_pool(name="sigp", bufs=2))
        statp = ctx.enter_context(tc.tile_pool(name="statp", bufs=2))
        diagp = ctx.enter_context(tc.tile_pool(name="diagp", bufs=10))
        hTp = ctx.enter_context(tc.tile_pool(name="hTp", bufs=3))
        uTp = ctx.enter_context(tc.tile_pool(name="uTp", bufs=3))
        gTp = ctx.enter_context(tc.tile_pool(name="gTp", bufs=3))
        thTp = ctx.enter_context(tc.tile_pool(name="thTp", bufs=3))
        lamTp = ctx.enter_context(tc.tile_pool(name="lamTp", bufs=3))
        bTp = ctx.enter_context(tc.tile_pool(name="bTp", bufs=3))
        sTp = ctx.enter_context(tc.tile_pool(name="sTp", bufs=3))
        sgTp = ctx.enter_context(tc.tile_pool(name="sgTp", bufs=3))
        owp = ctx.enter_context(tc.tile_pool(name="owp", bufs=3))

        # PSUM budget (8 banks): tps [128,1024] x1 = 2, uvps [128,512]
        # x2 = 2, preps [128,512] x2 = 2, yps [128,512] x2 = 2.
        tps = ctx.enter_context(tc.tile_pool(name="tps", bufs=1, space="PSUM"))
        uvps = ctx.enter_context(tc.tile_pool(name="uvps", bufs=2, space="PSUM"))
        preps = ctx.enter_context(tc.tile_pool(name="preps", bufs=2, space="PSUM"))
        yps = ctx.enter_context(tc.tile_pool(name="yps", bufs=2, space="PSUM"))

        # ---- constants ----
        identf = const.tile([128, 128], f32, tag="identf", name="identf")
        masks.make_identity(nc, identf[:])
        identb = const.tile([128, 128], fr, tag="identb", name="identb")
        nc.vector.tensor_copy(identb[:], identf[:])
        magic = const.tile([128, NTILE], i32, tag="magic", name="magic")
        nc.gpsimd.memset(magic[:], MAGIC)

        nw, bdt2 = [], []

        def load_vec_consts():
            bw = const.tile([128, 4], f32, tag="bdtw", name="bdtw")
            nc.sync.dma_start(bw[:], bdt_d.rearrange("(m p) -> p m", p=128))
            # scale in place: tanh(pre*0.5 + b_dt*0.5)
            nc.vector.tensor_scalar_mul(bw[:], bw[:], 0.5)
            for m in range(4):
                bdt2.append(bw[:, m:m + 1])

        # ---- load + transpose weights ----
        winT, wdtT, woutT = [], [], []
        wtiles = {}

        def win_dma():
            for k in range(2):
                t = const.tile([128, 1], f32, tag=f"nw{k}", name=f"nw{k}")
                nc.sync.dma_start(
                    t[:], nw_d[ts(k, 128)].rearrange("(a b) -> a b", b=1))
                nw.append(t)
            winw = wraw.tile([128, 8 * 256], f32, tag="winw", name="winw")
            for hf in range(2):
                nc.sync.dma_start(
                    winw[:, hf * 1024:(hf + 1) * 1024].rearrange(
                        "p (f d) -> p f d", f=4),
                    win_d[hf * 512:(hf + 1) * 512, :].rearrange(
                        "(f p) d -> p f d", p=128))
            wtiles["winw"] = winw

        def win_transform():
            # W_in [1024(feat), 256(d)] -> W_inT [2][128(d), 1024(feat)]
            winw = wtiles["winw"]
            win_raw = [winw[:, f * 256:(f + 1) * 256] for f in range(8)]
            for k in range(2):
                t = wT.tile([128, 1024], fr, tag=f"winT{k}", name=f"winT{k}")
                winT.append(t)
            for half in range(2):
                for k in range(2):
                    p = tps.tile([128, 2 * TC], f32, tag="tpx", name="tpw")
                    for j in range(4):
                        f = half * 4 + j
                        nc.tensor.matmul(p[:, ts(j, 128)],
                                         win_raw[f][:, ts(k, 128)], identf[:],
                                         is_transpose=True)
                    if half % 2 == 0:
                        nc.vector.tensor_copy(winT[k][:, ts(half, 512)],
                                              p[:, :512])
                    else:
                        nc.scalar.copy(winT[k][:, ts(half, 512)],
                                       p[:, :512])
            for k in range(2):
                # fold norm_w (per-d row scale) into W_inT
                nc.vector.tensor_scalar_mul(winT[k][:], winT[k][:], nw[k][:])

        def wdt_dma():
            wdtw = wraw.tile([128, 4 * 512], f32, tag="wdtw", name="wdtw")
            nc.sync.dma_start(
                wdtw[:].rearrange("p (m e) -> p m e", m=4),
                wdt_d.rearrange("(m p) e -> p m e", p=128))
            wtiles["wdtw"] = wdtw

        def wdt_transform():
            # W_dt [512(e_out), 512(e_in)] -> W_dtT [4][128(e_in), 512(e_out)]
            wdtw = wtiles["wdtw"]
            wdt_raw = [wdtw[:, m * 512:(m + 1) * 512] for m in range(4)]
            for k in range(4):
                t = wT.tile([128, 512], fr, tag=f"wdtT{k}", name=f"wdtT{k}")
                wdtT.append(t)
            for k in range(4):
                p = tps.tile([128, 2 * TC], f32, tag="tpx", name="tpw")
                for m in range(4):
                    nc.tensor.matmul(p[:, ts(m, 128)],
                                     wdt_raw[m][:, ts(k, 128)], identf[:],
                                     is_transpose=True)
                if k % 2 == 0:
                    nc.vector.tensor_copy(wdtT[k][:], p[:, :512])
                else:
                    nc.scalar.copy(wdtT[k][:], p[:, :512])

        def wout_dma():
            woutw = wraw.tile([128, 2 * 512], f32, tag="woutw", name="woutw")
            nc.sync.dma_start(
                woutw[:].rearrange("p (a e) -> p a e", a=2),
                wout_d.rearrange("(a p) e -> p a e", p=128))
            wtiles["woutw"] = woutw

        def wout_transform():
            # W_out [256(d), 512(e)] -> W_outT [4][128(e), 256(d)]
            woutw = wtiles["woutw"]
            wout_raw = [woutw[:, a * 512:(a + 1) * 512] for a in range(2)]
            for e in range(4):
                t = wT.tile([128, 256], fr, tag=f"woutT{e}", name=f"woutT{e}")
                woutT.append(t)
            for e in range(4):
                p = tps.tile([128, 2 * TC], f32, tag="tpx", name="tpw")
                for dd in range(2):
                    nc.tensor.matmul(p[:, ts(dd, 128)],
                                     wout_raw[dd][:, ts(e, 128)], identf[:],
                                     is_transpose=True)
                # fold the S = -2*s change of variable into W_out: scale by
                # -0.5 so sg = S*g needs no extra scaling (plain tensor_mul)
                if e % 2 == 0:
                    nc.vector.tensor_scalar_mul(woutT[e][:], p[:, :256], -0.5)
                else:
                    nc.scalar.mul(woutT[e][:], p[:, :256], -0.5)

        # ---- x load (bf16 cast-DMA), batched stats, one Newton chain ----
        xins = [None] * NCHUNK
        stat = {}

        def x_dma(h):
            """Load 2 chunks (pair h) of x as bf16 in one SWDGE cast DMA.
            One [128, 2*NTT*D] tile holds the pair; xins[c] are views."""
            pair = xinp.tile([128, 2 * NTT * D], fr, tag="xin", name="xin")
            for i in range(2):
                xins[2 * h + i] = pair[:, i * NTT * D:(i + 1) * NTT * D]
            c0 = 2 * h
            src = x_d[c0 * TC:(c0 + 2) * TC, :].rearrange(
                "(t p) d -> p t d", p=128)
            nc.gpsimd.dma_start(
                pair[:].rearrange("p (t d) -> p t d", t=2 * NTT), src)

        def stats(c):
            """RMS stats for chunk c: per token-tile Square + accum into
            vpk[:, 4c+t], split ACT/DVE (the two engines are near-tied;
            tensor_tensor_reduce hangs real hardware)."""
            xin = xins[c]
            for t in range(NTT):
                xv = xin[:, t * D:(t + 1) * D]
                col = stat["vpk"][:, c * NTT + t:c * NTT + t + 1]
                sq = sqp.tile([128, D], fr, tag="sq", name="sq")
                if t % 2 == 0:
                    nc.scalar.activation(sq[:], xv, AF.Square, accum_out=col)
                else:
                    nc.vector.scalar_tensor_tensor(
                        sq[:], xv, 0.0, xv, op0=ALU.add, op1=ALU.mult,
                        accum_out=col)

        def newton():
            """inv_rms = rsqrt(vpk/D + eps) for all 32 token-tiles at once
            (DVE only): bit-trick seed + 2 Newton iterations."""
            vpk = stat["vpk"]
            nv = statp.tile([128, NTILE], f32, tag="nv", name="nv")
            nc.vector.tensor_scalar(nv[:], vpk[:], 1.0 / D, EPS,
                                    op0=ALU.mult, op1=ALU.add)
            ny = statp.tile([128, NTILE], f32, tag="ny", name="ny")
            nyi = ny[:].bitcast(i32)
            nc.vector.tensor_scalar(nyi, nv[:].bitcast(i32), 1, None,
                                    op0=ALU.arith_shift_right)
            nc.vector.scalar_tensor_tensor(nyi, magic[:], 1, nyi,
                                           op0=ALU.bypass, op1=ALU.subtract)
            nt = statp.tile([128, NTILE], f32, tag="nt", name="nt")
            for _ in range(2):
                # t = v*y*y ; y = y * (1.5 - 0.5*t)
                nc.vector.tensor_mul(nt[:], ny[:], ny[:])
                nc.vector.tensor_mul(nt[:], nt[:], nv[:])
                nc.vector.tensor_scalar(nt[:], nt[:], -0.5, 1.5,
                                        op0=ALU.mult, op1=ALU.add)
                nc.vector.tensor_mul(ny[:], ny[:], nt[:])
            stat["ny"] = ny

        # ---- pipeline stages ----
        def trans_k(st, c, k):
            """Transpose-with-scale for chunk c, d-half k: for each token
            tile, hT = x_tile.T @ diag(inv_rms) on PE (one matmul each).
            diag tiles built on ACT (Copy, per-partition scale). Both
            d-halves share one [128, 2*TC] PSUM tile (one wide copy out)."""
            ny = stat["ny"]
            xin = xins[c]
            if k == 0:
                st["diag"] = []
                for t in range(NTT):
                    dg = diagp.tile([128, 128], fr, tag="dg", name="dg")
                    nc.scalar.activation(
                        dg[:], identb[:], AF.Copy,
                        scale=ny[:, c * NTT + t:c * NTT + t + 1])
                    st["diag"].append(dg)
                st["tpx"] = tps.tile([128, 2 * TC], f32, tag="tpx",
                                     name="tpx")
            p = st["tpx"]
            for t in range(NTT):
                nc.tensor.matmul(
                    p[:, k * TC + t * 128:k * TC + t * 128 + 128],
                    xin[:, t * D + k * 128:t * D + k * 128 + 128],
                    st["diag"][t][:])

        def trans_copies(st):
            hTw = hTp.tile([128, 2 * TC], fr, tag="hTw", name="hTw")
            nc.vector.tensor_copy(hTw[:], st["tpx"][:])
            st["hT"] = [hTw[:, :TC], hTw[:, TC:]]

        def front_in(st):
            """in_proj + silu (+ interleaved c+2 transposes)."""
            hT = st["hT"]
            uTw = [uTp.tile([128, 2 * TC], fr, tag=f"uTw{p}", name=f"uTw{p}")
                   for p in range(2)]
            gTw = [gTp.tile([128, 2 * TC], fr, tag=f"gTw{p}", name=f"gTw{p}")
                   for p in range(2)]
            uT = [uTw[m // 2][:, (m % 2) * TC:(m % 2 + 1) * TC]
                  for m in range(4)]
            gT = [gTw[m // 2][:, (m % 2) * TC:(m % 2 + 1) * TC]
                  for m in range(4)]
            st_T = st.get("st_T")
            for m in range(8):
                ps = uvps.tile([128, TC], f32, tag="uv", name="uv")
                for k in range(2):
                    nc.tensor.matmul(
                        ps[:], winT[k][:, ts(m, 128)], hT[k][:],
                        start=(k == 0), stop=(k == 1),
                    )
                dst = uT[m] if m < 4 else gT[m - 4]
                if silu_native:
                    nc.scalar.activation(dst, ps[:], AF.Silu)
                else:
                    # CoreSim has no Silu: decompose as x * sigmoid(x)
                    sig = sigp.tile([128, TC], f32, tag="sig", name="sig")
                    nc.scalar.activation(sig[:], ps[:], AF.Sigmoid)
                    nc.vector.tensor_mul(dst, ps[:], sig[:])
                # fill PE's silu-drain wait with the c+2 transposes
                if st_T is not None and m in (1, 3):
                    trans_k(st_T, st["c_T"], m // 2)
            st.update(uT=uT, gT=gT, gTw=gTw)

        def front_dt(st):
            """dt_proj + tanh + lam + b."""
            uT = st["uT"]
            thT = [thTp.tile([128, TC], fr, tag=f"thT{m}", name=f"thT{m}")
                   for m in range(4)]
            lamT = [lamTp.tile([128, TC], fr, tag=f"lamT{m}", name=f"lamT{m}")
                    for m in range(4)]
            bT = [bTp.tile([128, TC], fr, tag=f"bT{m}", name=f"bT{m}")
                  for m in range(4)]
            for m in range(4):
                ps = preps.tile([128, TC], f32, tag="pre", name="pre")
                for k in range(4):
                    nc.tensor.matmul(
                        ps[:], wdtT[k][:, ts(m, 128)], uT[k],
                        start=(k == 0), stop=(k == 3),
                    )
                nc.scalar.activation(thT[m][:], ps[:], AF.Tanh,
                                     bias=bdt2[m], scale=0.5)
                nc.vector.tensor_scalar(lamT[m][:], thT[m][:], 0.5, 0.5,
                                        op0=ALU.mult, op1=ALU.add)
                # b = (th - 1) * u in one DVE op
                nc.vector.scalar_tensor_tensor(
                    bT[m][:], thT[m][:], -1.0, uT[m],
                    op0=ALU.add, op1=ALU.mult)
            st.update(lamT=lamT, bT=bT)

        def scan_stage(st, sT_prev):
            # scan (DVE): S_t = lam_t*S_{t-1} + bT_t  => S = -2*s
            sTw = [sTp.tile([128, 2 * TC], fr, tag=f"sTw{p}",
                            name=f"sTw{p}") for p in range(2)]
            sgTw = [sgTp.tile([128, 2 * TC], fr, tag=f"sgTw{p}",
                              name=f"sgTw{p}") for p in range(2)]
            sT = [sTw[m // 2][:, (m % 2) * TC:(m % 2 + 1) * TC]
                  for m in range(4)]
            sgT = [sgTw[m // 2][:, (m % 2) * TC:(m % 2 + 1) * TC]
                   for m in range(4)]
            for m in range(4):
                init = 0.0 if sT_prev is None else sT_prev[m][:, TC - 1:TC]
                nc.vector.tensor_tensor_scan(
                    sT[m], st["lamT"][m][:], st["bT"][m][:], init,
                    op0=ALU.mult, op1=ALU.add,
                )
                # sg = S*g (the -0.5 of s = -S/2 is folded into woutT),
                # one wide op per m-pair
                if m % 2 == 1:
                    pr = m // 2
                    gw = st["gTw"][pr]
                    nc.vector.tensor_mul(sgTw[pr][:], sTw[pr][:], gw[:])
            st.update(sT=sT, sgT=sgT)
            return sT

        def out_pair(st, c, pair, split_store=False):
            # out_proj (sgT stationary -> y [tok,d] tiles) + residual into
            # the wide out tile.
            sgT = st["sgT"]
            xin = xins[c]
            if pair == 0:
                st["ow"] = owp.tile([128, NTT * D], f32, tag="ow", name="ow")
            ow = st["ow"]
            tt = (2 * pair, 2 * pair + 1)
            yp = yps.tile([128, 2 * D], f32, tag="y", name="y")
            for e in range(4):
                for i, t in enumerate(tt):
                    nc.tensor.matmul(
                        yp[:, i * D:(i + 1) * D],
                        sgT[e][:, ts(t, 128)], woutT[e][:],
                        start=(e == 0 and i == 0), stop=(e == 3 and i == 1),
                    )
            # residual add (DVE; Pool cannot access PSUM)
            nc.vector.tensor_add(
                ow[:, 2 * pair * D:(2 * pair + 2) * D], yp[:],
                xin[:, 2 * pair * D:(2 * pair + 2) * D])
            if split_store:
                t0 = c * TC + pair * 256
                dst = out_d[t0:t0 + 256, :].rearrange(
                    "(t p) d -> p t d", p=128)
                nc.sync.dma_start(
                    dst, ow[:, pair * 512:(pair + 1) * 512].rearrange(
                        "p (t d) -> p t d", t=2))

        def out_store(st, c):
            dst = out_d[c * TC:(c + 1) * TC, :].rearrange(
                "(t p) d -> p t d", p=128)
            nc.sync.dma_start(
                dst, st["ow"][:].rearrange("p (t d) -> p t d", t=NTT))

        # ---- emission ----
        # x DMAs lead: the x -> stats -> newton -> diag -> transpose chain
        # is the longest pole to the first in_proj; weights ride behind.
        sts = [dict() for _ in range(NCHUNK)]
        x_dma(0)
        x_dma(1)
        win_dma()
        x_dma(2)
        wdt_dma()
        x_dma(3)
        wout_dma()
        load_vec_consts()
        win_transform()
        wdt_transform()
        wout_transform()
        stat["vpk"] = statp.tile([128, NTILE], f32, tag="vpk", name="vpk")
        for c in range(NCHUNK):
            stats(c)
        newton()
        trans_k(sts[0], 0, 0)
        trans_k(sts[0], 0, 1)
        trans_copies(sts[0])
        trans_k(sts[1], 1, 0)
        trans_k(sts[1], 1, 1)
        trans_copies(sts[1])
        sts[0]["st_T"] = sts[2]
        sts[0]["c_T"] = 2
        front_in(sts[0])
        front_dt(sts[0])
        trans_copies(sts[2])
        sT_prev = None
        for c in range(NCHUNK):
            sT_prev = scan_stage(sts[c], sT_prev)
            last = c == NCHUNK - 1
            if c + 1 < NCHUNK:
                # transposes for c+3 ride inside front(c+1)'s stall slots
                if c + 3 < NCHUNK:
                    sts[c + 1]["st_T"] = sts[c + 3]
                    sts[c + 1]["c_T"] = c + 3
                front_in(sts[c + 1])
                out_pair(sts[c], c, 0, split_store=last)
                front_dt(sts[c + 1])
                out_pair(sts[c], c, 1, split_store=last)
                if c + 3 < NCHUNK:
                    trans_copies(sts[c + 3])
            else:
                out_pair(sts[c], c, 0, split_store=True)
                out_pair(sts[c], c, 1, split_store=True)
            if not last:
                out_store(sts[c], c)
            sts[c].clear()


def _emit_tiny(tc, aps):
    """Minimal program (timing control): one load + one store."""
    import contextlib
    nc = tc.nc
    import concourse.mybir as mybir
    f32 = mybir.dt.float32
    ctx = contextlib.ExitStack()
    with ctx:
        p = ctx.enter_context(tc.tile_pool(name="tiny", bufs=1))
        t = p.tile([128, 2], f32, tag="tiny", name="tiny")
        nc.sync.dma_start(t[:], aps["norm_w"].rearrange("(p m) -> p m", p=128))
        nc.sync.dma_start(
            aps["out"][0:1, :].rearrange("a (p m) -> p (a m)", p=128), t[:])


def build(mm_f32r=True, silu_native=True, repeats=1, **emit_kw):
    """Build and compile the Bass module (cached)."""
    key = ("nc", mm_f32r, silu_native, repeats,
           tuple(sorted(emit_kw.items())))
    if key in _CACHE:
        return _CACHE[key]

    from concourse import bacc, mybir, tile

    f32 = mybir.dt.float32
    nc = bacc.Bacc(
        "TRN2",
        target_bir_lowering=False,
        debug=False,
        num_devices=N_CORES,
    )
    aps = {
        "x": nc.dram_tensor("x", [K, D], f32, kind="ExternalInput").ap(),
        "norm_w": nc.dram_tensor("norm_w", [D], f32, kind="ExternalInput").ap(),
        "W_in": nc.dram_tensor("W_in", [2 * INNER, D], f32, kind="ExternalInput").ap(),
        "W_dt": nc.dram_tensor("W_dt", [INNER, INNER], f32, kind="ExternalInput").ap(),
        "b_dt": nc.dram_tensor("b_dt", [INNER], f32, kind="ExternalInput").ap(),
        "W_out": nc.dram_tensor("W_out", [D, INNER], f32, kind="ExternalInput").ap(),
        "out": nc.dram_tensor("out", [K, D], f32, kind="ExternalOutput").ap(),
    }
    with tile.TileContext(nc) as tc:
        if repeats == 0:
            _emit_tiny(tc, aps)
        for _ in range(repeats):
            _emit(tc, aps, mm_f32r=mm_f32r, silu_native=silu_native,
                  **emit_kw)
    nc.compile()
    _CACHE[key] = nc
    return nc


def make_in_maps(inputs):
    x = np.asarray(inputs["x"], dtype=np.float32)
    shared = {
        "norm_w": np.asarray(inputs["norm_w"], dtype=np.float32),
        "W_in": np.asarray(inputs["W_in"], dtype=np.float32),
        "W_dt": np.asarray(inputs["W_dt"], dtype=np.float32),
        "b_dt": np.asarray(inputs["b_dt"], dtype=np.float32),
        "W_out": np.asarray(inputs["W_out"], dtype=np.float32),
    }
    return [
        {"x": np.ascontiguousarray(x[b]), **shared} for b in range(N_CORES)
    ]


def run(inputs, trace=False, mm_f32r=True, silu_native=True, **kw):
    from concourse.bass_utils import run_bass_kernel_spmd

    nc = build(mm_f32r=mm_f32r, silu_native=silu_native)
    in_maps = make_in_maps(inputs)
    res = run_bass_kernel_spmd(
        nc, in_maps, core_ids=list(range(N_CORES)), trace=trace, **kw
    )
    out = np.stack([res.results[b]["out"] for b in range(N_CORES)], axis=0)
    return out, res


def kernel(**inputs) -> np.ndarray:
    out, _ = run(inputs, trace=False)
    return out
